# revision 16
# baseline (speedup 1.0000x reference)
"""Causal attention (B=4, S=2048, D=1024, fp32) on 8 Trainium2 NeuronCores.

Sharding: data-parallel over batch (4) x query-split (2) per batch. The two
cores of a batch take interleaved query rows (even/odd within each 512-row
super-block), which makes the causal workload identical on every core and
lets one SPMD program serve all 8 cores; the only per-core differences are
pure data (which query columns of x^T each core receives, and the mask
tiles, which carry the even/odd offset).

Per core:
  qT = (x_own @ W_q)^T, kT = (x @ W_k)^T, v = x @ W_v    (fp32r matmuls)
  For each of 4 query slots s (256 queries from super-block [512s, 512s+512)):
    for key block kb in [0, 4s+4): scoresT = kT_blk^T q  -> +mask -> exp
      (no max-subtraction: scaled scores are ~N(0,1), exp is fp32-safe)
      denominators via ones-matmul; ctx accumulation in PSUM
    normalize by reciprocal(denom), DMA out.

All matmuls use fp32r (full fp32 storage, ~tf32 matmul precision, bf16-class
throughput on the PE).
"""

import numpy as np

B, S, D = 4, 2048, 1024
NE = D // 128          # contraction chunks (d on partitions)
NKBLK = S // 128       # 128-wide key blocks
NSLOT = 4              # query slots per core
QW = 256               # queries per slot
OWNQ = NSLOT * QW      # 1024 queries per core
MASK_NEG = -1.0e30
SCALE = 1.0 / 32.0     # 1/sqrt(D)

_cached = {}


def _build():
    import concourse.bacc as bacc
    import concourse.tile as tile
    import concourse.mybir as mybir

    F32 = mybir.dt.float32
    F32R = mybir.dt.float32r
    EXP = mybir.ActivationFunctionType.Exp

    nc = bacc.Bacc("TRN2", target_bir_lowering=False, debug=False, num_devices=8,
                   dynamic_dma_scratch_size=2048)

    xt_d = nc.dram_tensor("xt", [D, S], F32R, kind="ExternalInput")
    xq_d = nc.dram_tensor("xq", [D, OWNQ], F32R, kind="ExternalInput")
    wq_d = nc.dram_tensor("wq", [D, D], F32R, kind="ExternalInput")
    wk_d = nc.dram_tensor("wk", [D, D], F32R, kind="ExternalInput")
    wv_d = nc.dram_tensor("wv", [D, D], F32R, kind="ExternalInput")
    mask_d = nc.dram_tensor("masks", [128, 4 * QW], F32, kind="ExternalInput")
    ones_d = nc.dram_tensor("ones", [128, 2], F32R, kind="ExternalInput")
    o_d = nc.dram_tensor("o", [OWNQ, D], F32, kind="ExternalOutput")

    with tile.TileContext(nc) as tc:
        with tc.tile_pool(name="res", bufs=1) as res:
            kT = []
            for c in range(NE):
                t = res.tile([128, S], F32R, name=f"kT{c}", tag=f"kT{c}")
                kT.append(t)
            vv = []
            for j in range(NKBLK):
                t = res.tile([128, D], F32R, name=f"v{j}", tag=f"v{j}")
                vv.append(t)
            qT = []
            for c in range(NE):
                t = res.tile([128, OWNQ], F32R, name=f"qT{c}", tag=f"qT{c}")
                qT.append(t)
            # ---------------- projection phase ----------------
            with (
                tc.tile_pool(name="wpool", bufs=2) as wpool,
                tc.tile_pool(name="xsp", bufs=3) as xsp,
                tc.tile_pool(name="pp", bufs=6, space="PSUM") as pp,
            ):
                def load_w_half(src, col0):
                    # [d, 512]-wide half of a weight matrix, all 8 d-chunks.
                    # Chunked DMAs spread across queues; issue split over the
                    # two HWDGE engines plus idle SWDGE to cut serial issue.
                    w_t = wpool.tile([128, NE * 512], F32R, name="w_t", tag="w")
                    for dc in range(NE):
                        eng = nc.sync if dc % 2 == 0 else nc.scalar
                        eng.dma_start(
                            w_t[:, dc * 512:(dc + 1) * 512],
                            src[dc * 128:(dc + 1) * 128, col0:col0 + 512],
                        )
                    return w_t

                def load_x_slice(src, col0):
                    xs_t = xsp.tile([128, NE * QW], F32R, name="xs_t", tag="xs")
                    for dc in range(NE):
                        eng = nc.sync if dc % 2 == 0 else nc.scalar
                        eng.dma_start(
                            xs_t[:, dc * QW:(dc + 1) * QW],
                            src[dc * 128:(dc + 1) * 128, col0:col0 + QW],
                        )
                    return xs_t

                # Combined K+V pass: one stream over xt computes both
                # kT e-half h2 and v d-out half h2, halving x re-reads.
                xs_first = None
                for h2 in range(2):
                    wk_t = load_w_half(wk_d, h2 * 512)
                    if h2 == 0:
                        # First slice before wv so the first matmuls' deps
                        # (wk + xs0) win the DMA bandwidth race at startup.
                        xs_first = load_x_slice(xt_d, 0)
                    wv_t = load_w_half(wv_d, h2 * 512)
                    for js in range(S // QW):
                        if h2 == 0 and js == 0:
                            xs_t = xs_first
                        else:
                            xs_t = load_x_slice(xt_d, js * QW)
                        for ei in range(4):
                            et = 4 * h2 + ei
                            ps = pp.tile([128, 512], F32, name="ps_p", tag="ps_p")
                            for dc in range(NE):
                                nc.tensor.matmul(
                                    ps[:, 0:QW],
                                    wk_t[:, dc * 512 + ei * 128: dc * 512 + (ei + 1) * 128],
                                    xs_t[:, dc * QW:(dc + 1) * QW],
                                    start=(dc == 0), stop=(dc == NE - 1),
                                )
                            nc.scalar.copy(kT[et][:, js * QW:(js + 1) * QW], ps[:, 0:QW])
                        for jt in range(QW // 128):
                            jc = (QW // 128) * js + jt
                            ps = pp.tile([128, 512], F32, name="ps_p", tag="ps_p")
                            for dc in range(NE):
                                nc.tensor.matmul(
                                    ps[:, 0:512],
                                    xs_t[:, dc * QW + jt * 128: dc * QW + jt * 128 + 128],
                                    wv_t[:, dc * 512:(dc + 1) * 512],
                                    start=(dc == 0), stop=(dc == NE - 1),
                                )
                            nc.vector.tensor_copy(
                                vv[jc][:, h2 * 512:(h2 + 1) * 512], ps[:, 0:512]
                            )

                # Q pass: qT[e, i] = sum_d Wq[d, e] xq[d, i]   (e-halves)
                for eh in range(2):
                    w_t = load_w_half(wq_d, eh * 512)
                    for isl in range(OWNQ // QW):
                        xs_t = load_x_slice(xq_d, isl * QW)
                        for ei in range(4):
                            et = 4 * eh + ei
                            ps = pp.tile([128, 512], F32, name="ps_p", tag="ps_p")
                            for dc in range(NE):
                                nc.tensor.matmul(
                                    ps[:, 0:QW],
                                    w_t[:, dc * 512 + ei * 128: dc * 512 + (ei + 1) * 128],
                                    xs_t[:, dc * QW:(dc + 1) * QW],
                                    start=(dc == 0), stop=(dc == NE - 1),
                                )
                            nc.scalar.copy(qT[et][:, isl * QW:(isl + 1) * QW], ps[:, 0:QW])

            # ---------------- attention phase ----------------
            with (
                tc.tile_pool(name="cns", bufs=1) as cns,
                tc.tile_pool(name="ptp", bufs=4) as ptp,
                tc.tile_pool(name="obp", bufs=2) as obp,
                tc.tile_pool(name="rcp", bufs=2) as rcp,
                tc.tile_pool(name="scp", bufs=3, space="PSUM") as scp,
                tc.tile_pool(name="ctxp", bufs=1, space="PSUM") as ctxp,
                tc.tile_pool(name="dnp", bufs=1, space="PSUM") as dnp,
            ):
                mask_t = cns.tile([128, 4 * QW], F32, name="mask_t", tag="mask_t")
                ones_t = cns.tile([128, 2], F32R, name="ones_t", tag="ones_t")
                nc.sync.dma_start(mask_t[:, :], mask_d[:, :])
                nc.sync.dma_start(ones_t[:, :], ones_d[:, :])
                def consume(item):
                    s, kb, pt, ctx, dn = item
                    nk = 4 * s + 4
                    for c in range(2):
                        # Both column groups live in one PSUM bank; start=True
                        # clears the whole bank, so only the first group may
                        # set it — the second lands on freshly cleared psum
                        # (has_written=0) and still overwrites, not adds.
                        nc.tensor.matmul(
                            dn[:, 2 * c:2 * c + 2],
                            pt[:, c * 128:(c + 1) * 128],
                            ones_t[:, :],
                            start=(kb == 0 and c == 0), stop=(kb == nk - 1),
                            skip_group_check=True,
                        )
                    for c in range(2):
                        for dh in range(2):
                            nc.tensor.matmul(
                                ctx[(c, dh)][:, :],
                                pt[:, c * 128:(c + 1) * 128],
                                vv[kb][:, dh * 512:(dh + 1) * 512],
                                start=(kb == 0), stop=(kb == nk - 1),
                            )
                    if kb == nk - 1:
                        rc = rcp.tile([128, 2], F32, name="rc", tag="rc")
                        nc.vector.reciprocal(rc[:, :], dn[:, 0:4:2])
                        for c in range(2):
                            ob = obp.tile([128, D], F32, name="ob", tag="ob")
                            for dh in range(2):
                                nc.vector.tensor_scalar_mul(
                                    ob[:, dh * 512:(dh + 1) * 512],
                                    ctx[(c, dh)][:, :],
                                    rc[:, c:c + 1],
                                )
                            nc.sync.dma_start(
                                o_d[s * QW + c * 128: s * QW + (c + 1) * 128, :],
                                ob[:, :],
                            )

                from collections import deque
                pending = deque()
                DEPTH = 2
                for s in range(NSLOT):
                    nk = 4 * s + 4
                    # Drain before each slot: the slot's ctx/dn pool slots
                    # (bufs=1) can only be re-allocated once the previous
                    # slot's normalize has been emitted.
                    while pending:
                        consume(pending.popleft())
                    ctx_cur = {}
                    for c in range(2):
                        for dh in range(2):
                            t = ctxp.tile(
                                [128, 512], F32,
                                name=f"ctx{c}{dh}", tag=f"ctx{c}{dh}",
                            )
                            ctx_cur[(c, dh)] = t
                    dn_cur = dnp.tile([128, 4], F32, name="dn", tag="dn")
                    for kb in range(nk):
                        ps_sc = scp.tile([128, QW], F32, name="ps_sc", tag="sc")
                        for ec in range(NE):
                            nc.tensor.matmul(
                                ps_sc[:, :],
                                kT[ec][:, kb * 128:(kb + 1) * 128],
                                qT[ec][:, s * QW:(s + 1) * QW],
                                start=(ec == 0), stop=(ec == NE - 1),
                            )
                        t_idx = kb - (nk - 4)
                        if t_idx >= 0:
                            nc.vector.tensor_add(
                                ps_sc[:, :], ps_sc[:, :],
                                mask_t[:, t_idx * QW:(t_idx + 1) * QW],
                            )
                        pt = ptp.tile([128, QW], F32R, name="pt", tag="pt")
                        nc.scalar.activation(pt[:, :], ps_sc[:, :], EXP, scale=SCALE)
                        pending.append((s, kb, pt, ctx_cur, dn_cur))
                        if len(pending) > DEPTH:
                            consume(pending.popleft())
                while pending:
                    consume(pending.popleft())

    nc.compile()
    return nc


def _get_nc():
    if "nc" not in _cached:
        _cached["nc"] = _build()
    return _cached["nc"]


def kernel(x, W_q, W_k, W_v):
    from concourse.bass_utils import run_bass_kernel_spmd

    x = np.asarray(x, dtype=np.float32)
    wq = np.ascontiguousarray(np.asarray(W_q, dtype=np.float32))
    wk = np.ascontiguousarray(np.asarray(W_k, dtype=np.float32))
    wv = np.ascontiguousarray(np.asarray(W_v, dtype=np.float32))
    ones = np.ones((128, 2), dtype=np.float32)

    p = np.arange(128, dtype=np.int64)[:, None]
    f = np.arange(QW, dtype=np.int64)[None, :]
    masks_h = []
    for h in range(2):
        tiles = [
            np.where(128 * t + p <= 2 * f + h, np.float32(0.0), np.float32(MASK_NEG))
            for t in range(4)
        ]
        masks_h.append(np.concatenate(tiles, axis=1).astype(np.float32))

    xt_b = [np.ascontiguousarray(x[b].T) for b in range(B)]
    in_maps = []
    for c in range(8):
        b, h = c // 2, c % 2
        xq = np.ascontiguousarray(x[b, h::2, :].T)
        in_maps.append({
            "xt": xt_b[b],
            "xq": xq,
            "wq": wq,
            "wk": wk,
            "wv": wv,
            "masks": masks_h[h],
            "ones": ones,
        })

    nc = _get_nc()
    res = run_bass_kernel_spmd(nc, in_maps, core_ids=list(range(8)))

    out = np.empty((B, S, D), dtype=np.float32)
    for c in range(8):
        b, h = c // 2, c % 2
        out[b, h::2, :] = res.results[c]["o"]
    return out


# revision 17
# speedup vs baseline: 1.0963x; 1.0963x over previous
"""Causal attention (B=4, S=2048, D=1024, fp32) on 8 Trainium2 NeuronCores.

Sharding: data-parallel over batch (4) x query-split (2) per batch. The two
cores of a batch take interleaved query rows (even/odd within each 512-row
super-block), which makes the causal workload identical on every core and
lets one SPMD program serve all 8 cores; the only per-core differences are
pure data (which query columns of x^T each core receives, and the mask
tiles, which carry the even/odd offset).

Per core:
  qT = (x_own @ W_q)^T, kT = (x @ W_k)^T, v = x @ W_v    (fp32r matmuls)
  For each of 4 query slots s (256 queries from super-block [512s, 512s+512)):
    for key block kb in [0, 4s+4): scoresT = kT_blk^T q  -> +mask -> exp
      (no max-subtraction: scaled scores are ~N(0,1), exp is fp32-safe)
      denominators via ones-matmul; ctx accumulation in PSUM
    normalize by reciprocal(denom), DMA out.

All matmuls use fp32r (full fp32 storage, ~tf32 matmul precision, bf16-class
throughput on the PE).
"""

import numpy as np

B, S, D = 4, 2048, 1024
NE = D // 128          # contraction chunks (d on partitions)
NKBLK = S // 128       # 128-wide key blocks
NSLOT = 4              # query slots per core
QW = 256               # queries per slot
OWNQ = NSLOT * QW      # 1024 queries per core
MASK_NEG = -1.0e30
SCALE = 1.0 / 32.0     # 1/sqrt(D)

_cached = {}


def _build():
    import concourse.bacc as bacc
    import concourse.tile as tile
    import concourse.mybir as mybir

    F32 = mybir.dt.float32
    F32R = mybir.dt.float32r
    EXP = mybir.ActivationFunctionType.Exp

    nc = bacc.Bacc("TRN2", target_bir_lowering=False, debug=False, num_devices=8,
                   dynamic_dma_scratch_size=2048)

    xt_d = nc.dram_tensor("xt", [D, S], F32R, kind="ExternalInput")
    xq_d = nc.dram_tensor("xq", [D, OWNQ], F32R, kind="ExternalInput")
    wq_d = nc.dram_tensor("wq", [D, D], F32R, kind="ExternalInput")
    wk_d = nc.dram_tensor("wk", [D, D], F32R, kind="ExternalInput")
    wv_d = nc.dram_tensor("wv", [D, D], F32R, kind="ExternalInput")
    mask_d = nc.dram_tensor("masks", [128, 4 * QW], F32, kind="ExternalInput")
    ones_d = nc.dram_tensor("ones", [128, 2], F32R, kind="ExternalInput")
    o_d = nc.dram_tensor("o", [OWNQ, D], F32, kind="ExternalOutput")

    with tile.TileContext(nc) as tc:
        with tc.tile_pool(name="res", bufs=1) as res:
            kT = []
            for c in range(NE):
                t = res.tile([128, S], F32R, name=f"kT{c}", tag=f"kT{c}")
                kT.append(t)
            vv = []
            for j in range(NKBLK):
                t = res.tile([128, D], F32R, name=f"v{j}", tag=f"v{j}")
                vv.append(t)
            qT = []
            for c in range(NE):
                t = res.tile([128, OWNQ], F32R, name=f"qT{c}", tag=f"qT{c}")
                qT.append(t)
            # ---------------- projection phase ----------------
            with (
                tc.tile_pool(name="wpool", bufs=2) as wpool,
                tc.tile_pool(name="xsp", bufs=3) as xsp,
                tc.tile_pool(name="pp", bufs=6, space="PSUM") as pp,
            ):
                def load_w_half(src, col0):
                    # [d, 512]-wide half of a weight matrix, all 8 d-chunks.
                    # Chunked DMAs spread across queues; issue split over the
                    # two HWDGE engines plus idle SWDGE to cut serial issue.
                    w_t = wpool.tile([128, NE * 512], F32R, name="w_t", tag="w")
                    for dc in range(NE):
                        eng = nc.sync if dc % 2 == 0 else nc.scalar
                        eng.dma_start(
                            w_t[:, dc * 512:(dc + 1) * 512],
                            src[dc * 128:(dc + 1) * 128, col0:col0 + 512],
                        )
                    return w_t

                def load_x_slice(src, col0):
                    xs_t = xsp.tile([128, NE * QW], F32R, name="xs_t", tag="xs")
                    for dc in range(NE):
                        eng = nc.sync if dc % 2 == 0 else nc.scalar
                        eng.dma_start(
                            xs_t[:, dc * QW:(dc + 1) * QW],
                            src[dc * 128:(dc + 1) * 128, col0:col0 + QW],
                        )
                    return xs_t

                # Combined K+V pass: one stream over xt computes both
                # kT e-half h2 and v d-out half h2, halving x re-reads.
                for h2 in range(2):
                    wk_t = load_w_half(wk_d, h2 * 512)
                    wv_t = load_w_half(wv_d, h2 * 512)
                    for js in range(S // QW):
                        xs_t = load_x_slice(xt_d, js * QW)
                        for ei in range(4):
                            et = 4 * h2 + ei
                            ps = pp.tile([128, 512], F32, name="ps_p", tag="ps_p")
                            for dc in range(NE):
                                nc.tensor.matmul(
                                    ps[:, 0:QW],
                                    wk_t[:, dc * 512 + ei * 128: dc * 512 + (ei + 1) * 128],
                                    xs_t[:, dc * QW:(dc + 1) * QW],
                                    start=(dc == 0), stop=(dc == NE - 1),
                                )
                            nc.scalar.copy(kT[et][:, js * QW:(js + 1) * QW], ps[:, 0:QW])
                        for jt in range(QW // 128):
                            jc = (QW // 128) * js + jt
                            ps = pp.tile([128, 512], F32, name="ps_p", tag="ps_p")
                            for dc in range(NE):
                                nc.tensor.matmul(
                                    ps[:, 0:512],
                                    xs_t[:, dc * QW + jt * 128: dc * QW + jt * 128 + 128],
                                    wv_t[:, dc * 512:(dc + 1) * 512],
                                    start=(dc == 0), stop=(dc == NE - 1),
                                )
                            nc.vector.tensor_copy(
                                vv[jc][:, h2 * 512:(h2 + 1) * 512], ps[:, 0:512]
                            )

                # Q pass: qT[e, i] = sum_d Wq[d, e] xq[d, i]   (e-halves)
                for eh in range(2):
                    w_t = load_w_half(wq_d, eh * 512)
                    for isl in range(OWNQ // QW):
                        xs_t = load_x_slice(xq_d, isl * QW)
                        for ei in range(4):
                            et = 4 * eh + ei
                            ps = pp.tile([128, 512], F32, name="ps_p", tag="ps_p")
                            for dc in range(NE):
                                nc.tensor.matmul(
                                    ps[:, 0:QW],
                                    w_t[:, dc * 512 + ei * 128: dc * 512 + (ei + 1) * 128],
                                    xs_t[:, dc * QW:(dc + 1) * QW],
                                    start=(dc == 0), stop=(dc == NE - 1),
                                )
                            nc.scalar.copy(qT[et][:, isl * QW:(isl + 1) * QW], ps[:, 0:QW])

            # ---------------- attention phase ----------------
            with (
                tc.tile_pool(name="cns", bufs=1) as cns,
                tc.tile_pool(name="ptp", bufs=4) as ptp,
                tc.tile_pool(name="obp", bufs=2) as obp,
                tc.tile_pool(name="rcp", bufs=2) as rcp,
                tc.tile_pool(name="scp", bufs=3, space="PSUM") as scp,
                tc.tile_pool(name="ctxp", bufs=1, space="PSUM") as ctxp,
                tc.tile_pool(name="dnp", bufs=1, space="PSUM") as dnp,
            ):
                mask_t = cns.tile([128, 4 * QW], F32, name="mask_t", tag="mask_t")
                ones_t = cns.tile([128, 2], F32R, name="ones_t", tag="ones_t")
                nc.sync.dma_start(mask_t[:, :], mask_d[:, :])
                nc.sync.dma_start(ones_t[:, :], ones_d[:, :])
                def consume(item):
                    s, kb, pt, ctx, dn = item
                    nk = 4 * s + 4
                    for c in range(2):
                        # Both column groups live in one PSUM bank; start=True
                        # clears the whole bank, so only the first group may
                        # set it — the second lands on freshly cleared psum
                        # (has_written=0) and still overwrites, not adds.
                        nc.tensor.matmul(
                            dn[:, 2 * c:2 * c + 2],
                            pt[:, c * 128:(c + 1) * 128],
                            ones_t[:, :],
                            start=(kb == 0 and c == 0), stop=(kb == nk - 1),
                            skip_group_check=True,
                        )
                    for c in range(2):
                        for dh in range(2):
                            nc.tensor.matmul(
                                ctx[(c, dh)][:, :],
                                pt[:, c * 128:(c + 1) * 128],
                                vv[kb][:, dh * 512:(dh + 1) * 512],
                                start=(kb == 0), stop=(kb == nk - 1),
                            )
                    if kb == nk - 1:
                        rc = rcp.tile([128, 2], F32, name="rc", tag="rc")
                        nc.vector.reciprocal(rc[:, :], dn[:, 0:4:2])
                        for c in range(2):
                            ob = obp.tile([128, D], F32, name="ob", tag="ob")
                            for dh in range(2):
                                nc.vector.tensor_scalar_mul(
                                    ob[:, dh * 512:(dh + 1) * 512],
                                    ctx[(c, dh)][:, :],
                                    rc[:, c:c + 1],
                                )
                            nc.sync.dma_start(
                                o_d[s * QW + c * 128: s * QW + (c + 1) * 128, :],
                                ob[:, :],
                            )

                from collections import deque
                pending = deque()
                DEPTH = 2
                for s in range(NSLOT):
                    nk = 4 * s + 4
                    # Drain before each slot: the slot's ctx/dn pool slots
                    # (bufs=1) can only be re-allocated once the previous
                    # slot's normalize has been emitted.
                    while pending:
                        consume(pending.popleft())
                    ctx_cur = {}
                    for c in range(2):
                        for dh in range(2):
                            t = ctxp.tile(
                                [128, 512], F32,
                                name=f"ctx{c}{dh}", tag=f"ctx{c}{dh}",
                            )
                            ctx_cur[(c, dh)] = t
                    dn_cur = dnp.tile([128, 4], F32, name="dn", tag="dn")
                    for kb in range(nk):
                        ps_sc = scp.tile([128, QW], F32, name="ps_sc", tag="sc")
                        for ec in range(NE):
                            nc.tensor.matmul(
                                ps_sc[:, :],
                                kT[ec][:, kb * 128:(kb + 1) * 128],
                                qT[ec][:, s * QW:(s + 1) * QW],
                                start=(ec == 0), stop=(ec == NE - 1),
                            )
                        t_idx = kb - (nk - 4)
                        if t_idx >= 0:
                            nc.vector.tensor_add(
                                ps_sc[:, :], ps_sc[:, :],
                                mask_t[:, t_idx * QW:(t_idx + 1) * QW],
                            )
                        pt = ptp.tile([128, QW], F32R, name="pt", tag="pt")
                        nc.scalar.activation(pt[:, :], ps_sc[:, :], EXP, scale=SCALE)
                        pending.append((s, kb, pt, ctx_cur, dn_cur))
                        if len(pending) > DEPTH:
                            consume(pending.popleft())
                while pending:
                    consume(pending.popleft())

    nc.compile()
    return nc


def _get_nc():
    if "nc" not in _cached:
        _cached["nc"] = _build()
    return _cached["nc"]


def kernel(x, W_q, W_k, W_v):
    from concourse.bass_utils import run_bass_kernel_spmd

    x = np.asarray(x, dtype=np.float32)
    wq = np.ascontiguousarray(np.asarray(W_q, dtype=np.float32))
    wk = np.ascontiguousarray(np.asarray(W_k, dtype=np.float32))
    wv = np.ascontiguousarray(np.asarray(W_v, dtype=np.float32))
    ones = np.ones((128, 2), dtype=np.float32)

    p = np.arange(128, dtype=np.int64)[:, None]
    f = np.arange(QW, dtype=np.int64)[None, :]
    masks_h = []
    for h in range(2):
        tiles = [
            np.where(128 * t + p <= 2 * f + h, np.float32(0.0), np.float32(MASK_NEG))
            for t in range(4)
        ]
        masks_h.append(np.concatenate(tiles, axis=1).astype(np.float32))

    xt_b = [np.ascontiguousarray(x[b].T) for b in range(B)]
    in_maps = []
    for c in range(8):
        b, h = c // 2, c % 2
        xq = np.ascontiguousarray(x[b, h::2, :].T)
        in_maps.append({
            "xt": xt_b[b],
            "xq": xq,
            "wq": wq,
            "wk": wk,
            "wv": wv,
            "masks": masks_h[h],
            "ones": ones,
        })

    nc = _get_nc()
    res = run_bass_kernel_spmd(nc, in_maps, core_ids=list(range(8)))

    out = np.empty((B, S, D), dtype=np.float32)
    for c in range(8):
        b, h = c // 2, c % 2
        out[b, h::2, :] = res.results[c]["o"]
    return out
